# revision 1
# baseline (speedup 1.0000x reference)
"""DSAutoCorrelation Trainium2 kernel.

Math (verified vs reference, rel err ~8e-7 in numpy):
  C = H*E = 512 channels, L = 2048, B = 16, top_k = 7.
  R[b,l]    = sum_t <k[b,t,:], q[b,(t+l)%L,:]>_c      (= C * mean_value[b,l])
  topk over mean_b R -> 7 delays d_k; w[b,:] = softmax(R[b,d]/C)
  out[b,l,:] = sum_k w[b,k] * v[b,(l+d_k)%L,:]

Device split (8 cores, 2 batches each):
  K1: D[b,p,u] = sum_{i<16, c} K^T[c,128i+p] * Q^T[c,(128i+u)%L]  (pure PE matmul)
      host: R[b,l] = sum_p D[b,p,(p+l)%L]  (tiny reindex) -> topk -> softmax
  K2: rolled weighted sum of V^T via dynamic-offset DVE taps, PE-transpose back
      to natural [l,c] layout, DMA out.
"""

import numpy as np

B, L, H, E = 16, 2048, 8, 64
C = H * E
NCORES = 8
BPC = B // NCORES
TOPK = 7  # int(math.log(2048))
NB = L // 128  # 16 row-blocks

_CACHE = {}


def _f32r():
    from concourse import mybir
    return mybir.dt.float32r


def _build_k1():
    from concourse import bacc, mybir
    from concourse.tile import TileContext

    f32 = mybir.dt.float32
    f32r = mybir.dt.float32r
    nc = bacc.Bacc("TRN2", target_bir_lowering=False, debug=False, num_devices=NCORES)
    qt = nc.dram_tensor("qt", (BPC, C, L), f32r, kind="ExternalInput")
    kt = nc.dram_tensor("kt", (BPC, C, L), f32r, kind="ExternalInput")
    Dout = nc.dram_tensor("D", (BPC, 128, L), f32, kind="ExternalOutput")

    with TileContext(nc) as tc:
        with (
            tc.tile_pool(name="qk", bufs=2) as qkpool,
            tc.tile_pool(name="ps", bufs=2, space="PSUM") as pspool,
            tc.tile_pool(name="dsb", bufs=4) as dpool,
        ):
            for b in range(BPC):
                kts = []
                qts = []
                for cb in range(4):
                    kt_t = qkpool.tile([128, L], f32r, tag=f"kt{cb}", name=f"kt{cb}")
                    nc.sync.dma_start(kt_t[:], kt[b, 128 * cb:128 * (cb + 1), :])
                    kts.append(kt_t)
                    qt_t = qkpool.tile([128, L], f32r, tag=f"qt{cb}", name=f"qt{cb}")
                    nc.sync.dma_start(qt_t[:], qt[b, 128 * cb:128 * (cb + 1), :])
                    qts.append(qt_t)

                psums = [pspool.tile([128, 512], f32, tag=f"ps{u}", name=f"ps{u}") for u in range(4)]
                first = [True] * 4
                for i in range(NB):
                    for cb in range(4):
                        lhs = kts[cb][:, 128 * i:128 * (i + 1)]
                        for u in range(4):
                            u0 = 512 * u
                            s = (128 * i + u0) % L
                            last = (i == NB - 1) and (cb == 3)
                            if s + 512 <= L:
                                nc.tensor.matmul(
                                    psums[u][:, 0:512], lhs, qts[cb][:, s:s + 512],
                                    start=first[u], stop=last)
                            else:
                                n1 = L - s
                                nc.tensor.matmul(
                                    psums[u][:, 0:n1], lhs, qts[cb][:, s:L],
                                    start=first[u], stop=False)
                                nc.tensor.matmul(
                                    psums[u][:, n1:512], lhs, qts[cb][:, 0:512 - n1],
                                    start=first[u], stop=last)
                            first[u] = False
                for u in range(4):
                    d_sb = dpool.tile([128, 512], f32, tag="dsb", name="dsb")
                    nc.vector.tensor_copy(d_sb[:], psums[u][:])
                    nc.sync.dma_start(Dout[b, :, 512 * u:512 * (u + 1)], d_sb[:])
    nc.compile()
    return nc


def _build_k2():
    from concourse import bacc, bass, mybir
    from concourse.tile import TileContext

    f32 = mybir.dt.float32
    i32 = mybir.dt.int32
    nc = bacc.Bacc("TRN2", target_bir_lowering=False, debug=False, num_devices=NCORES)
    vns = [nc.dram_tensor(f"v{b}", (L, C), f32, kind="ExternalInput")
           for b in range(BPC)]
    wb = nc.dram_tensor("wb", (BPC, 128, TOPK), f32, kind="ExternalInput")
    gidx = nc.dram_tensor("gidx", (128, NB * TOPK), i32, kind="ExternalInput")
    out = nc.dram_tensor("out", (BPC, L, C), f32, kind="ExternalOutput")

    with TileContext(nc) as tc:
        with (
            tc.tile_pool(name="consts", bufs=1) as cpool,
            tc.tile_pool(name="taps", bufs=6) as tappool,
            tc.tile_pool(name="acc", bufs=4) as accpool,
        ):
            gi_sb = cpool.tile([128, NB * TOPK], i32, name="gi_sb")
            nc.sync.dma_start(gi_sb[:], gidx[:, :])
            w_sbs = []
            for b in range(BPC):
                w_sb = cpool.tile([128, TOPK], f32, tag=f"w{b}", name=f"w{b}")
                nc.sync.dma_start(w_sb[:], wb[b, :, :])
                w_sbs.append(w_sb)
            for b in range(BPC):
                for m in range(NB):
                    tap = tappool.tile([128, TOPK * C], f32, tag="tap", name="tap")
                    for k in range(TOPK):
                        nc.gpsimd.indirect_dma_start(
                            out=tap[:, C * k:C * (k + 1)],
                            out_offset=None,
                            in_=vns[b][:, :],
                            in_offset=bass.IndirectOffsetOnAxis(
                                ap=gi_sb[:, m * TOPK + k:m * TOPK + k + 1], axis=0),
                        )
                    acc = accpool.tile([128, C], f32, tag="acc", name="acc")
                    nc.vector.tensor_scalar(
                        acc[:], tap[:, 0:C], w_sbs[b][:, 0:1], None,
                        mybir.AluOpType.mult)
                    for k in range(1, TOPK):
                        nc.vector.scalar_tensor_tensor(
                            acc[:], tap[:, C * k:C * (k + 1)],
                            w_sbs[b][:, k:k + 1], acc[:],
                            mybir.AluOpType.mult, mybir.AluOpType.add)
                    nc.sync.dma_start(out[b, 128 * m:128 * (m + 1), :], acc[:])
    nc.compile()
    return nc


def _get_kernels():
    if "k1" not in _CACHE:
        _CACHE["k1"] = _build_k1()
        _CACHE["k2"] = _build_k2()
    return _CACHE["k1"], _CACHE["k2"]


_DIAG_P = np.arange(128)[:, None]
_DIAG_IDX = (np.arange(128)[:, None] + np.arange(L)[None, :]) % L


def kernel(queries, keys, values, attn_mask=None, _trace=False):
    from concourse import bass_utils

    k1, k2 = _get_kernels()
    q = np.ascontiguousarray(np.asarray(queries, dtype=np.float32).reshape(B, L, C).transpose(0, 2, 1))
    kk = np.ascontiguousarray(np.asarray(keys, dtype=np.float32).reshape(B, L, C).transpose(0, 2, 1))
    v = np.ascontiguousarray(np.asarray(values, dtype=np.float32).reshape(B, L, C))

    in1 = [{"qt": q[BPC * r:BPC * (r + 1)], "kt": kk[BPC * r:BPC * (r + 1)]}
           for r in range(NCORES)]
    res1 = bass_utils.run_bass_kernel_spmd(
        k1, in1, core_ids=list(range(NCORES)), trace=_trace)
    D = np.concatenate([r["D"] for r in res1.results], axis=0)  # [B, 128, L]

    R = D[:, _DIAG_P, _DIAG_IDX].sum(axis=1)  # [B, L]
    mean_value = R / C
    didx = np.argsort(-mean_value.mean(axis=0), kind="stable")[:TOPK]
    wlog = mean_value[:, didx]
    wexp = np.exp(wlog - wlog.max(axis=1, keepdims=True))
    w = (wexp / wexp.sum(axis=1, keepdims=True)).astype(np.float32)  # [B, TOPK]

    wb = np.ascontiguousarray(np.repeat(w[:, None, :], 128, axis=1))  # [B,128,TOPK]
    # gidx[p, m*TOPK+k] = (128m + p + d_k) % L
    p_ = np.arange(128)[:, None]
    mk = (128 * (np.arange(NB * TOPK) // TOPK))[None, :] + didx[np.arange(NB * TOPK) % TOPK][None, :]
    gidx = ((p_ + mk) % L).astype(np.int32)
    gidx = np.ascontiguousarray(gidx)
    in2 = [{"v0": v[BPC * r], "v1": v[BPC * r + 1], "wb": wb[BPC * r:BPC * (r + 1)],
            "gidx": gidx} for r in range(NCORES)]
    res2 = bass_utils.run_bass_kernel_spmd(
        k2, in2, core_ids=list(range(NCORES)), trace=_trace)
    out = np.concatenate([r["out"] for r in res2.results], axis=0)  # [B, L, C]
    if _trace:
        kernel._last_trace = (res1, res2)
    return out.reshape(B, L, H, E).astype(np.float32)



# revision 8
# speedup vs baseline: 2.2589x; 2.2589x over previous
"""DSAutoCorrelation Trainium2 kernel.

Math (verified vs reference, rel err ~8e-7 in numpy):
  C = H*E = 512 channels, L = 2048, B = 16, top_k = 7.
  R[b,l]    = sum_t <k[b,t,:], q[b,(t+l)%L,:]>_c      (= C * mean_value[b,l])
  topk over mean_b R -> 7 delays d_k; w[b,:] = softmax(R[b,d]/C)
  out[b,l,:] = sum_k w[b,k] * v[b,(l+d_k)%L,:]

Device split (8 cores, 2 batches each):
  K1: D[b,p,u] = sum_{i<16, c} K^T[c,128i+p] * Q^T[c,(128i+u)%L]  (pure PE matmul)
      cb-outer loop order for load/compute overlap; qt kept doubled in SBUF
      so no matmul ever wraps (wrap splits cost 4 cyc/col on short f32r pieces).
      host: R[b,l] = sum_p D[b,p,(p+l)%L]  (tiny reindex) -> topk -> softmax
  K2: delays are known on host before K2 is built, so the roll becomes a
      STATIC free-axis offset on transposed v ([C, L] per batch): no gather.
      fp16 throughout (tolerance 2e-2; fp16 keeps err ~1e-3): 2x DVE rate,
      half DMA. Tiles split between DVE (scalar_tensor_tensor accumulate)
      and PE (shift-matmul with w*I stationary, PSUM-accumulated, ACT copy).
"""

import numpy as np

B, L, H, E = 16, 2048, 8, 64
C = H * E
NCORES = 8
BPC = B // NCORES
TOPK = 7  # int(math.log(2048))
NB = L // 128  # 16 row-blocks

_CACHE = {}


def _build_k1():
    from concourse import bacc, mybir
    from concourse.tile import TileContext

    f32 = mybir.dt.float32
    f32r = mybir.dt.float32r
    nc = bacc.Bacc("TRN2", target_bir_lowering=False, debug=False, num_devices=NCORES)
    qt = nc.dram_tensor("qt", (BPC, C, L), f32r, kind="ExternalInput")
    kt = nc.dram_tensor("kt", (BPC, C, L), f32r, kind="ExternalInput")
    Dout = nc.dram_tensor("D", (BPC, 128, L), f32, kind="ExternalOutput")

    with TileContext(nc) as tc:
        with (
            tc.tile_pool(name="ktp", bufs=4) as ktpool,
            tc.tile_pool(name="qtp", bufs=4) as qtpool,
            tc.tile_pool(name="ps", bufs=2, space="PSUM") as pspool,
            tc.tile_pool(name="dsb", bufs=4) as dpool,
        ):
            for b in range(BPC):
                psums = [pspool.tile([128, 512], f32, tag=f"ps{u}", name=f"ps{u}")
                         for u in range(4)]
                for cb in range(4):
                    kt_t = ktpool.tile([128, L], f32r, tag="kt", name="kt")
                    nc.sync.dma_start(kt_t[:], kt[b, 128 * cb:128 * (cb + 1), :])
                    qt_t = qtpool.tile([128, 2 * L], f32r, tag="qt", name="qt")
                    nc.sync.dma_start(qt_t[:, 0:L], qt[b, 128 * cb:128 * (cb + 1), :])
                    nc.sync.dma_start(qt_t[:, L:2 * L], qt[b, 128 * cb:128 * (cb + 1), :])
                    for i in range(NB):
                        lhs = kt_t[:, 128 * i:128 * (i + 1)]
                        for u in range(4):
                            s = 128 * i + 512 * u
                            nc.tensor.matmul(
                                psums[u][:, 0:512], lhs, qt_t[:, s:s + 512],
                                start=(cb == 0 and i == 0),
                                stop=(cb == 3 and i == NB - 1))
                for u in range(4):
                    d_sb = dpool.tile([128, 512], f32, tag="dsb", name="dsb")
                    nc.vector.tensor_copy(d_sb[:], psums[u][:])
                    nc.sync.dma_start(Dout[b, :, 512 * u:512 * (u + 1)], d_sb[:])
    nc.compile()
    return nc


# (b, cb) tiles aggregated on PE via shift-matmul; the rest on DVE.
_PE_TILES = {(0, 1), (0, 3), (1, 1)}


def _build_k2(didx):
    from concourse import bacc, mybir
    from concourse.tile import TileContext

    f32 = mybir.dt.float32
    f16 = mybir.dt.float16
    mult = mybir.AluOpType.mult
    add = mybir.AluOpType.add
    nc = bacc.Bacc("TRN2", target_bir_lowering=False, debug=False, num_devices=NCORES)
    vt = nc.dram_tensor("vt", (BPC, C, L), f16, kind="ExternalInput")
    wb = nc.dram_tensor("wb", (BPC, 128, TOPK), f32, kind="ExternalInput")
    wi = nc.dram_tensor("wi", (BPC, 128, TOPK * 128), f16, kind="ExternalInput")
    outT = nc.dram_tensor("out", (BPC, C, L), f16, kind="ExternalOutput")

    with TileContext(nc) as tc:
        with (
            tc.tile_pool(name="consts", bufs=1) as cpool,
            tc.tile_pool(name="vtp", bufs=3) as vtpool,
            tc.tile_pool(name="acc", bufs=2) as accpool,
            tc.tile_pool(name="st", bufs=2) as stpool,
            tc.tile_pool(name="ps", bufs=2, space="PSUM") as pspool,
        ):
            wb_ts, wi_ts = [], []
            for b in range(BPC):
                wb_t = cpool.tile([128, TOPK], f32, tag=f"wb{b}", name=f"wb{b}")
                nc.sync.dma_start(wb_t[:], wb[b, :, :])
                wb_ts.append(wb_t)
                wi_t = cpool.tile([128, TOPK * 128], f16, tag=f"wi{b}", name=f"wi{b}")
                nc.sync.dma_start(wi_t[:], wi[b, :, :])
                wi_ts.append(wi_t)
            for b in range(BPC):
                for cb in range(4):
                    v_t = vtpool.tile([128, L], f16, tag="vt", name="vt")
                    nc.sync.dma_start(v_t[:], vt[b, 128 * cb:128 * (cb + 1), :])
                    if (b, cb) in _PE_TILES:
                        psums = [pspool.tile([128, 512], f32, tag=f"pp{u}", name=f"pp{u}")
                                 for u in range(4)]
                        for u in range(4):
                            # start=True zeroes the whole PSUM bank, so the
                            # sole start must be a full-width (non-wrapping)
                            # tap; the rest accumulate with start=False.
                            ks = list(range(TOPK))
                            k0 = next(k for k in ks if (512 * u + didx[k]) % L <= L - 512)
                            order = [k0] + [k for k in ks if k != k0]
                            for j, k in enumerate(order):
                                lhsT = wi_ts[b][:, 128 * k:128 * (k + 1)]
                                s = (512 * u + didx[k]) % L
                                n1 = min(512, L - s)
                                last = (j == len(order) - 1)
                                nc.tensor.matmul(
                                    psums[u][:, 0:n1], lhsT, v_t[:, s:s + n1],
                                    start=(j == 0), stop=last)
                                if n1 < 512:
                                    nc.tensor.matmul(
                                        psums[u][:, n1:512], lhsT, v_t[:, 0:512 - n1],
                                        start=False, stop=last)
                        st = stpool.tile([128, L], f16, tag="st", name="st")
                        for u in range(4):
                            nc.scalar.copy(st[:, 512 * u:512 * (u + 1)], psums[u][:])
                        nc.sync.dma_start(outT[b, 128 * cb:128 * (cb + 1), :], st[:])
                    else:
                        # scalar_tensor_tensor has no fast DVE mode; plain
                        # tensor_scalar runs 4x and tensor_tensor 2x on packed
                        # fp16, so scale taps into tmps and tree-add instead.
                        ta = accpool.tile([128, L], f16, tag="ta", name="ta")
                        tb = accpool.tile([128, L], f16, tag="tb", name="tb")
                        for k in range(TOPK):
                            d = int(didx[k])
                            n1 = L - d
                            w_ap = wb_ts[b][:, k:k + 1]
                            dst = ta if k == 0 else tb
                            nc.vector.tensor_scalar(
                                dst[:, 0:n1], v_t[:, d:L], w_ap, None, mult)
                            if d:
                                nc.vector.tensor_scalar(
                                    dst[:, n1:L], v_t[:, 0:d], w_ap, None, mult)
                            if k > 0:
                                nc.vector.tensor_tensor(ta[:], ta[:], tb[:], add)
                        nc.sync.dma_start(outT[b, 128 * cb:128 * (cb + 1), :], ta[:])
    nc.compile()
    return nc


def _get_k1():
    if "k1" not in _CACHE:
        _CACHE["k1"] = _build_k1()
    return _CACHE["k1"]


def _get_k2(didx):
    key = ("k2", didx)
    if key not in _CACHE:
        _CACHE[key] = _build_k2(didx)
    return _CACHE[key]


_DIAG_P = np.arange(128)[:, None]
_DIAG_IDX = (np.arange(128)[:, None] + np.arange(L)[None, :]) % L


def kernel(queries, keys, values, attn_mask=None, _trace=False):
    from concourse import bass_utils

    k1 = _get_k1()
    q = np.ascontiguousarray(
        np.asarray(queries, dtype=np.float32).reshape(B, L, C).transpose(0, 2, 1))
    kk = np.ascontiguousarray(
        np.asarray(keys, dtype=np.float32).reshape(B, L, C).transpose(0, 2, 1))
    vt16 = np.ascontiguousarray(
        np.asarray(values, dtype=np.float32).reshape(B, L, C).transpose(0, 2, 1)
        .astype(np.float16))

    in1 = [{"qt": q[BPC * r:BPC * (r + 1)], "kt": kk[BPC * r:BPC * (r + 1)]}
           for r in range(NCORES)]
    res1 = bass_utils.run_bass_kernel_spmd(
        k1, in1, core_ids=list(range(NCORES)), trace=_trace)
    D = np.concatenate([r["D"] for r in res1.results], axis=0)  # [B, 128, L]

    R = D[:, _DIAG_P, _DIAG_IDX].sum(axis=1)  # [B, L]
    mean_value = R / C
    didx = np.argsort(-mean_value.mean(axis=0), kind="stable")[:TOPK]
    wlog = mean_value[:, didx]
    wexp = np.exp(wlog - wlog.max(axis=1, keepdims=True))
    w = (wexp / wexp.sum(axis=1, keepdims=True)).astype(np.float32)  # [B, TOPK]

    k2 = _get_k2(tuple(int(d) for d in didx))
    wb16 = np.ascontiguousarray(
        np.repeat(w[:, None, :], 128, axis=1).astype(np.float32))  # [B,128,TOPK]
    eye = np.eye(128, dtype=np.float32)
    wi16 = np.ascontiguousarray(
        (w[:, :, None, None] * eye).transpose(0, 2, 1, 3)
        .reshape(B, 128, TOPK * 128).astype(np.float16))  # [B,128,TOPK*128]

    in2 = [{"vt": vt16[BPC * r:BPC * (r + 1)], "wb": wb16[BPC * r:BPC * (r + 1)],
            "wi": wi16[BPC * r:BPC * (r + 1)]} for r in range(NCORES)]
    res2 = bass_utils.run_bass_kernel_spmd(
        k2, in2, core_ids=list(range(NCORES)), trace=_trace)
    outT = np.concatenate([r["out"] for r in res2.results], axis=0)  # [B, C, L] f16
    if _trace:
        kernel._last_trace = (res1, res2)
    return np.ascontiguousarray(
        outT.transpose(0, 2, 1)).astype(np.float32).reshape(B, L, H, E)
